# revision 2
# baseline (speedup 1.0000x reference)
"""Trainium2 Bass kernel for the ActionVQVAE forward pass.

Data-parallel across 8 NeuronCores: the batch (131072 rows) is split into 8
shards of 16384 rows; all weights are replicated. Each core runs encoder MLP ->
VQ argmin -> gather -> loss partials. The decoder is precomputed once per core
on the 512 codebook rows (its input is always a codebook row), so the per-row
decoder collapses into the same gather that produces q_st.

Outputs per core: idx shard (int32), q_st shard (f32), and per-partition loss
partials which the host sums into the scalar total_loss during unsharding.
"""

import numpy as np

B_TOTAL = 131072
A = 6            # action dim
H = 256          # hidden
D = 128          # latent
KCB = 512        # codebook size
NCORES = 8
BLOC = B_TOTAL // NCORES   # rows per core
RB = 512                   # rows per block
PCH = 128                  # rows per chunk (psum partition dim)
NCH = RB // PCH            # chunks per block
RPAD = 16                  # padded recons width (6 used)
TW = D + RPAD              # gather-table row width (f32 elements)
BETA = 0.25

_NC_CACHE = {}


def _build_nc(bloc, enc_f32r=False, vq_f32r=False):
    from concourse import bacc, bass, mybir, tile

    f32 = mybir.dt.float32
    f32r = mybir.dt.float32r
    u32 = mybir.dt.uint32
    i32 = mybir.dt.int32
    AX = mybir.AxisListType
    AF = mybir.ActivationFunctionType
    OP = mybir.AluOpType

    nblk = bloc // RB
    nc = bacc.Bacc(None, target_bir_lowering=False)

    def dp(name, shape, dt=f32, out=False):
        return nc.declare_dram_parameter(name, list(shape), dt, isOutput=out)

    actT_d = dp("actT", (A, bloc))
    act_d = dp("act", (bloc, A))
    w1T_d = dp("w1T", (A, H))
    b1_d = dp("b1", (H,))
    w2T_d = dp("w2T", (H, H))
    b2_d = dp("b2", (H,))
    muT_d = dp("muT", (H, D))
    bm_d = dp("bm", (D,))
    embT_d = dp("embT", (D, KCB))
    embN_d = dp("embN", (KCB, D))
    d1T_d = dp("d1T", (D, H))
    bd1_d = dp("bd1", (H,))
    d2T_d = dp("d2T", (H, H))
    bd2_d = dp("bd2", (H,))
    rT_d = dp("rT", (H, RPAD))
    br_d = dp("br", (RPAD,))
    idx_d = dp("idx_o", (bloc,), i32, True)
    q_d = dp("q_o", (bloc, D), f32, True)
    par_d = dp("partials", (PCH, 3), f32, True)

    def enc_cast(ap):
        return ap.bitcast(f32r) if enc_f32r else ap

    def vq_cast(ap):
        return ap.bitcast(f32r) if vq_f32r else ap

    with tile.TileContext(nc) as tc:
        with (
            tc.tile_pool(name="wp", bufs=1) as wp,
            tc.tile_pool(name="av", bufs=2) as av,
            tc.tile_pool(name="scr", bufs=1) as scr,
            tc.tile_pool(name="stg", bufs=1) as stg,
            tc.tile_pool(name="pm", bufs=1, space="PSUM") as pm,
            tc.tile_pool(name="dr", bufs=1, space="DRAM") as dr,
        ):
            table = dr.tile([KCB, TW], f32)

            # ---------- preamble: weights into SBUF ----------
            w1T = wp.tile([A, H], f32)
            nc.sync.dma_start(w1T[:], w1T_d[:])
            w2T = []
            muT = []
            d2T = []
            rT = []
            for j in range(2):
                t = wp.tile([PCH, H], f32, name=f"w2T{j}")
                nc.sync.dma_start(t[:], w2T_d[j * 128:(j + 1) * 128, :])
                w2T.append(t)
                t = wp.tile([PCH, D], f32, name=f"muT{j}")
                nc.sync.dma_start(t[:], muT_d[j * 128:(j + 1) * 128, :])
                muT.append(t)
                t = wp.tile([PCH, H], f32, name=f"d2T{j}")
                nc.sync.dma_start(t[:], d2T_d[j * 128:(j + 1) * 128, :])
                d2T.append(t)
                t = wp.tile([PCH, RPAD], f32, name=f"rT{j}")
                nc.sync.dma_start(t[:], rT_d[j * 128:(j + 1) * 128, :])
                rT.append(t)
            embT = wp.tile([D, KCB], f32)
            nc.sync.dma_start(embT[:], embT_d[:])
            d1T = wp.tile([D, H], f32)
            nc.sync.dma_start(d1T[:], d1T_d[:])

            def bias_tiles(dram, n, nm):
                out = []
                for j in range(n):
                    t = wp.tile([PCH, 1], f32, name=f"{nm}{j}")
                    nc.sync.dma_start(t[:], dram[j * 128:(j + 1) * 128, None])
                    out.append(t)
                return out

            b1t = bias_tiles(b1_d, 2, "b1t")
            b2t = bias_tiles(b2_d, 2, "b2t")
            bd1t = bias_tiles(bd1_d, 2, "bd1t")
            bd2t = bias_tiles(bd2_d, 2, "bd2t")
            bmt = wp.tile([D, 1], f32)
            nc.sync.dma_start(bmt[:], bm_d[:, None])
            brt = wp.tile([RPAD, 1], f32)
            nc.sync.dma_start(brt[:], br_d[:, None])

            ones_col = wp.tile([D, 1], f32)
            nc.gpsimd.memset(ones_col[:], 1.0)
            ones_row = wp.tile([1, PCH], f32)
            nc.gpsimd.memset(ones_row[:], 1.0)

            # ---------- -0.5*|emb_k|^2 replicated across partitions ----------
            sqE = scr.tile([D, KCB], f32)
            nc.scalar.activation(sqE[:], embT[:], AF.Square)
            esq_ps = pm.tile([1, KCB], f32, tag="pre")
            nc.tensor.matmul(esq_ps[:], ones_col[:], sqE[:], start=True, stop=True)
            embsq_row = wp.tile([1, KCB], f32)
            nc.scalar.activation(embsq_row[:], esq_ps[:], AF.Copy, scale=-0.5)
            ebc_ps = pm.tile([PCH, KCB], f32, tag="pre")
            nc.tensor.matmul(ebc_ps[:], ones_row[:], embsq_row[:], start=True, stop=True)
            embsqF = wp.tile([PCH, KCB], f32)
            nc.scalar.activation(embsqF[:], ebc_ps[:], AF.Copy)

            # ---------- decoder precompute on the 512 codebook rows ----------
            t1 = []
            for j in range(2):
                ps = pm.tile([PCH, KCB], f32, tag="pre")
                nc.tensor.matmul(ps[:], d1T[:, j * 128:(j + 1) * 128], embT[:],
                                 start=True, stop=True)
                t = wp.tile([PCH, KCB], f32, name=f"t1_{j}")
                nc.scalar.activation(t[:], ps[:], AF.Relu, bias=bd1t[j][:])
                t1.append(t)
            t2 = []
            for j in range(2):
                ps = pm.tile([PCH, KCB], f32, tag="pre")
                for k in range(2):
                    nc.tensor.matmul(ps[:], d2T[k][:, j * 128:(j + 1) * 128], t1[k][:],
                                     start=(k == 0), stop=(k == 1))
                t = wp.tile([PCH, KCB], f32, name=f"t2_{j}")
                nc.scalar.activation(t[:], ps[:], AF.Relu, bias=bd2t[j][:])
                t2.append(t)
            rec_ps = pm.tile([RPAD, KCB], f32, tag="pre")
            for k in range(2):
                nc.tensor.matmul(rec_ps[:], rT[k][:], t2[k][:],
                                 start=(k == 0), stop=(k == 1))
            recT = wp.tile([RPAD, KCB], f32)
            nc.scalar.activation(recT[:], rec_ps[:], AF.Tanh, bias=brt[:])

            # ---------- assemble gather table in DRAM ----------
            embB = scr.tile([PCH, KCB // PCH, D], f32)
            nc.sync.dma_start(embB[:], embN_d[:].rearrange("(c p) d -> p c d", p=PCH))
            nc.sync.dma_start(table[:, 0:D].rearrange("(c p) d -> p c d", p=PCH), embB[:])
            nc.sync.dma_start(table[:, D:TW].rearrange("k a -> a k"), recT[:])

            # ---------- staging ----------
            esq_st = stg.tile([PCH, nblk], f32)
            rsq_st = stg.tile([PCH, nblk], f32)
            mvr_st = stg.tile([PCH, nblk], f32)

            # ---------- main loop ----------
            for b in range(nblk):
                r0 = b * RB
                xT = av.tile([A, RB], f32, tag="xT", bufs=3)
                nc.sync.dma_start(xT[:], actT_d[:, r0:r0 + RB])
                actn = av.tile([PCH, NCH, A], f32, tag="actn", bufs=3)
                nc.sync.dma_start(
                    actn[:], act_d[r0:r0 + RB, :].rearrange("(c p) a -> p c a", p=PCH))

                h1 = []
                for j in range(2):
                    ps = pm.tile([PCH, RB], f32, tag=f"h1_{j}")
                    nc.tensor.matmul(ps[:], enc_cast(w1T[:, j * 128:(j + 1) * 128]),
                                     enc_cast(xT[:]), start=True, stop=True)
                    t = av.tile([PCH, RB], f32, tag=f"h1s_{j}")
                    nc.scalar.activation(t[:], ps[:], AF.Relu, bias=b1t[j][:])
                    h1.append(t)
                h2 = []
                for j in range(2):
                    ps = pm.tile([PCH, RB], f32, tag=f"h2_{j}")
                    for k in range(2):
                        nc.tensor.matmul(ps[:], enc_cast(w2T[k][:, j * 128:(j + 1) * 128]),
                                         enc_cast(h1[k][:]), start=(k == 0), stop=(k == 1))
                    t = av.tile([PCH, RB], f32, tag=f"h2s_{j}")
                    nc.scalar.activation(t[:], ps[:], AF.Relu, bias=b2t[j][:])
                    h2.append(t)
                encps = pm.tile([PCH, RB], f32, tag="enc")
                for k in range(2):
                    nc.tensor.matmul(encps[:], enc_cast(muT[k][:]), enc_cast(h2[k][:]),
                                     start=(k == 0), stop=(k == 1))
                encT = av.tile([PCH, RB], f32, tag="encT")
                nc.scalar.activation(encT[:], encps[:], AF.Identity, bias=bmt[:])
                sq_scr = scr.tile([PCH, RB], f32, tag="sqscr")
                nc.scalar.activation(sq_scr[:], encT[:], AF.Square,
                                     accum_out=esq_st[:, b:b + 1])

                # VQ: score = enc . e_k - 0.5*|e_k|^2 ; argmax == argmin dist
                mv = av.tile([PCH, NCH * 8], f32, tag="mv")   # col layout e*NCH+c
                ib = av.tile([PCH, NCH * 8], u32, tag="ib")
                for c in range(NCH):
                    sps = pm.tile([PCH, KCB], f32, tag="s", bufs=2)
                    nc.tensor.matmul(sps[:], vq_cast(encT[:, c * 128:(c + 1) * 128]),
                                     vq_cast(embT[:]), start=True, stop=True)
                    ssb = av.tile([PCH, KCB], f32, tag="ssb", bufs=3)
                    nc.vector.tensor_tensor(out=ssb[:], in0=sps[:], in1=embsqF[:],
                                            op=OP.add)
                    mv8 = mv[:].rearrange("p (e c) -> p c e", c=NCH)[:, c, :]
                    ib8 = ib[:].rearrange("p (e c) -> p c e", c=NCH)[:, c, :]
                    nc.vector.max(mv8, ssb[:])
                    nc.vector.max_index(ib8, mv8, ssb[:])
                nc.vector.tensor_reduce(out=mvr_st[:, b:b + 1], in_=mv[:, 0:NCH],
                                        axis=AX.X, op=OP.add)

                gt = av.tile([PCH, NCH, TW], f32, tag="gath", bufs=3)
                for c in range(NCH):
                    # HW indirect DMA supports one offset per partition
                    nc.gpsimd.indirect_dma_start(
                        out=gt[:, c, :], out_offset=None,
                        in_=table[:],
                        in_offset=bass.IndirectOffsetOnAxis(ap=ib[:, c:c + 1], axis=0),
                    )
                nc.sync.dma_start(
                    q_d[r0:r0 + RB, :].rearrange("(c p) d -> p c d", p=PCH),
                    gt[:, :, 0:D])
                nc.sync.dma_start(
                    idx_d[r0:r0 + RB].rearrange("(c p) -> p c", p=PCH),
                    ib[:, 0:NCH].bitcast(i32))

                t6 = scr.tile([PCH, NCH, A], f32, tag="t6")
                nc.vector.tensor_tensor(out=t6[:], in0=gt[:, :, D:D + A], in1=actn[:],
                                        op=OP.subtract)
                sq6 = scr.tile([PCH, NCH * A], f32, tag="sq6")
                nc.scalar.activation(sq6[:], t6[:].rearrange("p c a -> p (c a)"),
                                     AF.Square, accum_out=rsq_st[:, b:b + 1])

            # ---------- partials ----------
            pc = stg.tile([PCH, 3], f32)
            nc.vector.tensor_reduce(out=pc[:, 0:1], in_=esq_st[:], axis=AX.X, op=OP.add)
            nc.vector.tensor_reduce(out=pc[:, 1:2], in_=mvr_st[:], axis=AX.X, op=OP.add)
            nc.vector.tensor_reduce(out=pc[:, 2:3], in_=rsq_st[:], axis=AX.X, op=OP.add)
            nc.sync.dma_start(par_d[:], pc[:])

    nc.compile()
    return nc


def _make_in_map(core, action, enc1_w, enc1_b, enc2_w, enc2_b, mu_w, mu_b, emb,
                 dec1_w, dec1_b, dec2_w, dec2_b, rec_w, rec_b, bloc=BLOC):
    c = np.ascontiguousarray
    sl = slice(core * bloc, (core + 1) * bloc)
    act = c(action[sl]).astype(np.float32)
    rT = np.zeros((H, RPAD), np.float32)
    rT[:, :A] = rec_w.T
    br = np.zeros((RPAD,), np.float32)
    br[:A] = rec_b
    return {
        "actT": c(act.T),
        "act": act,
        "w1T": c(enc1_w.T),
        "b1": c(enc1_b),
        "w2T": c(enc2_w.T),
        "b2": c(enc2_b),
        "muT": c(mu_w.T),
        "bm": c(mu_b),
        "embT": c(emb.T),
        "embN": c(emb),
        "d1T": c(dec1_w.T),
        "bd1": c(dec1_b),
        "d2T": c(dec2_w.T),
        "bd2": c(dec2_b),
        "rT": rT,
        "br": br,
    }


def kernel(action, enc1_w, enc1_b, enc2_w, enc2_b, mu_w, mu_b, emb,
           dec1_w, dec1_b, dec2_w, dec2_b, rec_w, rec_b,
           _trace=False, _result_hook=None):
    from concourse.bass_utils import run_bass_kernel_spmd

    key = (BLOC, False, False)
    if key not in _NC_CACHE:
        _NC_CACHE[key] = _build_nc(*key)
    nc = _NC_CACHE[key]

    args = (action, enc1_w, enc1_b, enc2_w, enc2_b, mu_w, mu_b, emb,
            dec1_w, dec1_b, dec2_w, dec2_b, rec_w, rec_b)
    in_maps = [_make_in_map(core, *args) for core in range(NCORES)]
    res = run_bass_kernel_spmd(nc, in_maps, core_ids=list(range(NCORES)),
                               trace=_trace)
    if _result_hook is not None:
        _result_hook(res)

    idx = np.concatenate([res.results[c]["idx_o"] for c in range(NCORES)])
    q_st = np.concatenate([res.results[c]["q_o"] for c in range(NCORES)])
    par = np.stack([res.results[c]["partials"] for c in range(NCORES)])
    # partials: [:, :, 0]=sum(enc^2), [:, :, 1]=sum(maxval), [:, :, 2]=sum((rec-act)^2)
    esum = float(par[:, :, 0].astype(np.float64).sum())
    msum = float(par[:, :, 1].astype(np.float64).sum())
    rsum = float(par[:, :, 2].astype(np.float64).sum())
    vq_sum = esum - 2.0 * msum          # sum over rows of min squared distance
    recons_loss = rsum / (B_TOTAL * A)
    vq_loss = (1.0 + BETA) * vq_sum / (B_TOTAL * D)
    total_loss = np.float32(recons_loss + vq_loss)
    return idx.astype(np.int32), q_st.astype(np.float32), total_loss


# revision 9
# speedup vs baseline: 1.1201x; 1.1201x over previous
"""Trainium2 Bass kernel for the ActionVQVAE forward pass.

Data-parallel across 8 NeuronCores: the batch (131072 rows) is split into 8
shards of 16384 rows; all weights are replicated. Each core runs encoder MLP ->
VQ argmin -> gather -> loss partials. The decoder is precomputed once per core
on the 512 codebook rows (its input is always a codebook row), so the per-row
decoder collapses into the same gather that produces q_st.

Matmul precision mode: "f32" uses native fp32 matmuls (~8 cyc/row on HW);
"bf16x3" splits each operand into bf16 hi+lo and uses 3 bf16 products
(hi*hi + hi*lo + lo*hi, fp32 PSUM accumulate) for ~2^-17 relative error at
~2.7x the speed. The VQ argmin tolerates that error level (top-2 score gaps
are quantized by the reference's own fp32 rounding at ~1.5e-8).

Outputs per core: idx shard (int32), q_st shard (f32), and per-partition loss
partials which the host sums into the scalar total_loss during unsharding.
"""

import numpy as np

B_TOTAL = 131072
A = 6            # action dim
H = 256          # hidden
D = 128          # latent
KCB = 512        # codebook size
NCORES = 8
BLOC = B_TOTAL // NCORES   # rows per core
RB = 512                   # rows per block
PCH = 128                  # rows per chunk (psum partition dim)
NCH = RB // PCH            # chunks per block
RPAD = 16                  # padded recons width (6 used)
TW = D + RPAD              # gather-table row width (f32 elements)
BETA = 0.25

ENC_MODE = "bf16x3"        # "f32" | "bf16x3"
VQ_MODE = "bf16x3"         # "f32" | "bf16x3"

_NC_CACHE = {}


def _split_bf16(x):
    import ml_dtypes
    bf = ml_dtypes.bfloat16
    hi = x.astype(bf)
    lo = (x.astype(np.float32) - hi.astype(np.float32)).astype(bf)
    return hi, lo


def _build_nc(bloc, enc_mode=ENC_MODE, vq_mode=VQ_MODE):
    from concourse import bacc, bass, mybir, tile

    f32 = mybir.dt.float32
    bf16 = mybir.dt.bfloat16
    u32 = mybir.dt.uint32
    i32 = mybir.dt.int32
    AX = mybir.AxisListType
    AF = mybir.ActivationFunctionType
    OP = mybir.AluOpType

    nblk = bloc // RB
    nc = bacc.Bacc(None, target_bir_lowering=False)

    def dp(name, shape, dt=f32, out=False):
        return nc.declare_dram_parameter(name, list(shape), dt, isOutput=out)

    # activations input: f32 transposed plus (for bf16x3) pre-split copies
    act_d = dp("act", (bloc, A))
    if enc_mode == "bf16x3":
        actTh_d = dp("actTh", (A, bloc), bf16)
        actTl_d = dp("actTl", (A, bloc), bf16)
    else:
        actT_d = dp("actT", (A, bloc))

    def wparam(name, shape, mode):
        if mode == "bf16x3":
            return (dp(name + "h", shape, bf16), dp(name + "l", shape, bf16))
        return dp(name, shape)

    w1T_p = wparam("w1T", (A, H), enc_mode)
    w2T_p = wparam("w2T", (H, H), enc_mode)
    muT_p = wparam("muT", (H, D), enc_mode)
    embT_p = wparam("embT", (D, KCB), vq_mode)
    b1_d = dp("b1", (H,))
    b2_d = dp("b2", (H,))
    bm_d = dp("bm", (D,))
    embTf_d = dp("embTf", (D, KCB))     # f32 embT for table/embsq math
    embN_d = dp("embN", (KCB, D))
    d1T_d = dp("d1T", (D, H))
    bd1_d = dp("bd1", (H,))
    d2T_d = dp("d2T", (H, H))
    bd2_d = dp("bd2", (H,))
    rT_d = dp("rT", (H, RPAD))
    br_d = dp("br", (RPAD,))
    idx_d = dp("idx_o", (bloc,), i32, True)
    q_d = dp("q_o", (bloc, D), f32, True)
    par_d = dp("partials", (PCH, 3), f32, True)

    with tile.TileContext(nc) as tc:
        with (
            tc.tile_pool(name="wp", bufs=1) as wp,
            tc.tile_pool(name="av", bufs=2) as av,
            tc.tile_pool(name="scr", bufs=1) as scr,
            tc.tile_pool(name="stg", bufs=1) as stg,
            tc.tile_pool(name="pm", bufs=1, space="PSUM") as pm,
            tc.tile_pool(name="dr", bufs=1, space="DRAM") as dr,
        ):
            table = dr.tile([KCB, TW], f32)

            # ---------- preamble: weights into SBUF ----------
            def load_w(param, shape, mode, nm):
                """Load a weight, split into <=128-partition row chunks.
                Returns a list of chunk tiles (each f32, or (hi, lo) bf16)."""
                nch = (shape[0] + 127) // 128
                out = []
                for k in range(nch):
                    rs = slice(k * 128, min((k + 1) * 128, shape[0]))
                    cshape = [rs.stop - rs.start, shape[1]]
                    if mode == "bf16x3":
                        th = wp.tile(cshape, bf16, name=f"{nm}h{k}")
                        tl = wp.tile(cshape, bf16, name=f"{nm}l{k}")
                        nc.sync.dma_start(th[:], param[0][rs, :])
                        nc.sync.dma_start(tl[:], param[1][rs, :])
                        out.append((th, tl))
                    else:
                        t = wp.tile(cshape, f32, name=f"{nm}{k}")
                        nc.sync.dma_start(t[:], param[rs, :])
                        out.append(t)
                return out

            w1T = load_w(w1T_p, (A, H), enc_mode, "w1T")[0]
            w2T = load_w(w2T_p, (H, H), enc_mode, "w2T")   # 2 row chunks
            muT = load_w(muT_p, (H, D), enc_mode, "muT")   # 2 row chunks
            embT = load_w(embT_p, (D, KCB), vq_mode, "embT")[0]
            embTf = wp.tile([D, KCB], f32)
            nc.sync.dma_start(embTf[:], embTf_d[:])
            d1T = wp.tile([D, H], f32)
            nc.sync.dma_start(d1T[:], d1T_d[:])
            d2T = []
            rT = []
            for j in range(2):
                t = wp.tile([PCH, H], f32, name=f"d2T{j}")
                nc.sync.dma_start(t[:], d2T_d[j * 128:(j + 1) * 128, :])
                d2T.append(t)
                t = wp.tile([PCH, RPAD], f32, name=f"rT{j}")
                nc.sync.dma_start(t[:], rT_d[j * 128:(j + 1) * 128, :])
                rT.append(t)

            def bias_tiles(dram, n, nm):
                out = []
                for j in range(n):
                    t = wp.tile([PCH, 1], f32, name=f"{nm}{j}")
                    nc.sync.dma_start(t[:], dram[j * 128:(j + 1) * 128, None])
                    out.append(t)
                return out

            b1t = bias_tiles(b1_d, 2, "b1t")
            b2t = bias_tiles(b2_d, 2, "b2t")
            bd1t = bias_tiles(bd1_d, 2, "bd1t")
            bd2t = bias_tiles(bd2_d, 2, "bd2t")
            bmt = wp.tile([D, 1], f32)
            nc.sync.dma_start(bmt[:], bm_d[:, None])
            brt = wp.tile([RPAD, 1], f32)
            nc.sync.dma_start(brt[:], br_d[:, None])

            ones_col = wp.tile([D, 1], f32)
            nc.gpsimd.memset(ones_col[:], 1.0)
            ones2_bf = wp.tile([2, PCH], bf16)
            nc.gpsimd.memset(ones2_bf[:], 1.0)

            # ---------- score bias: -0.5*|emb_k|^2 as bf16 hi+lo rows ----------
            sqE = scr.tile([D, KCB], f32)
            nc.scalar.activation(sqE[:], embTf[:], AF.Square)
            esq_ps = pm.tile([1, KCB], f32, tag="pre")
            nc.tensor.matmul(esq_ps[:], ones_col[:], sqE[:], start=True, stop=True)
            embsq_row = wp.tile([1, KCB], f32)
            nc.scalar.activation(embsq_row[:], esq_ps[:], AF.Copy, scale=-0.5)
            bias_hi_row = scr.tile([1, KCB], bf16)
            nc.scalar.activation(bias_hi_row[:], embsq_row[:], AF.Copy)
            bias_lo_f = scr.tile([1, KCB], f32)
            nc.vector.tensor_tensor(out=bias_lo_f[:], in0=embsq_row[:],
                                    in1=bias_hi_row[:], op=OP.subtract)
            bias_lo_row = scr.tile([1, KCB], bf16)
            nc.vector.tensor_copy(out=bias_lo_row[:], in_=bias_lo_f[:])
            bias_hl = wp.tile([2, KCB], bf16)
            nc.sync.dma_start(bias_hl[0:1, :], bias_hi_row[:])
            nc.sync.dma_start(bias_hl[1:2, :], bias_lo_row[:])

            # ---------- decoder precompute on the 512 codebook rows ----------
            t1 = []
            for j in range(2):
                ps = pm.tile([PCH, KCB], f32, tag="pre")
                nc.tensor.matmul(ps[:], d1T[:, j * 128:(j + 1) * 128], embTf[:],
                                 start=True, stop=True)
                t = wp.tile([PCH, KCB], f32, name=f"t1_{j}")
                nc.scalar.activation(t[:], ps[:], AF.Relu, bias=bd1t[j][:])
                t1.append(t)
            t2 = []
            for j in range(2):
                ps = pm.tile([PCH, KCB], f32, tag="pre")
                for k in range(2):
                    nc.tensor.matmul(ps[:], d2T[k][:, j * 128:(j + 1) * 128], t1[k][:],
                                     start=(k == 0), stop=(k == 1))
                t = wp.tile([PCH, KCB], f32, name=f"t2_{j}")
                nc.scalar.activation(t[:], ps[:], AF.Relu, bias=bd2t[j][:])
                t2.append(t)
            rec_ps = pm.tile([RPAD, KCB], f32, tag="pre")
            for k in range(2):
                nc.tensor.matmul(rec_ps[:], rT[k][:], t2[k][:],
                                 start=(k == 0), stop=(k == 1))
            recT = wp.tile([RPAD, KCB], f32)
            nc.scalar.activation(recT[:], rec_ps[:], AF.Tanh, bias=brt[:])

            # ---------- assemble gather table in DRAM ----------
            embB = scr.tile([PCH, KCB // PCH, D], f32)
            nc.sync.dma_start(embB[:], embN_d[:].rearrange("(c p) d -> p c d", p=PCH))
            nc.sync.dma_start(table[:, 0:D].rearrange("(c p) d -> p c d", p=PCH), embB[:])
            nc.sync.dma_start(table[:, D:TW].rearrange("k a -> a k"), recT[:])

            # ---------- staging ----------
            esq_st = stg.tile([PCH, nblk], f32)
            rsq_st = stg.tile([PCH, nblk], f32)
            mvr_st = stg.tile([PCH, nblk], f32)

            # split-matmul product pairs: lhsT.T @ rhs for hi/lo components.
            # w and x are APs (f32 mode) or (hi_ap, lo_ap) tuples (bf16x3).
            def products(w, x, mode):
                if mode == "bf16x3":
                    return [(w[0], x[0]), (w[0], x[1]), (w[1], x[0])]
                return [(w, x)]

            # emit one PSUM accumulation group from a list of (lhsT, rhs) APs
            def emit_group(ps, pairs, extra_last=None):
                n = len(pairs) + (1 if extra_last is not None else 0)
                for i, (lw, rx) in enumerate(pairs):
                    nc.tensor.matmul(ps[:], lw, rx, start=(i == 0),
                                     stop=(i == n - 1))
                if extra_last is not None:
                    nc.tensor.matmul(ps[:], extra_last[0], extra_last[1],
                                     start=False, stop=True)

            # ---------- main loop ----------
            for b in range(nblk):
                r0 = b * RB
                if enc_mode == "bf16x3":
                    xh = av.tile([A, RB], bf16, tag="xh", bufs=3)
                    nc.sync.dma_start(xh[:], actTh_d[:, r0:r0 + RB])
                    xl = av.tile([A, RB], bf16, tag="xl", bufs=3)
                    nc.sync.dma_start(xl[:], actTl_d[:, r0:r0 + RB])
                    xT = (xh, xl)
                else:
                    xt = av.tile([A, RB], f32, tag="xT", bufs=3)
                    nc.sync.dma_start(xt[:], actT_d[:, r0:r0 + RB])
                    xT = xt
                actn = av.tile([PCH, NCH, A], f32, tag="actn", bufs=3)
                nc.sync.dma_start(
                    actn[:], act_d[r0:r0 + RB, :].rearrange("(c p) a -> p c a", p=PCH))

                def act_split(ps_tile, func, bias_ap, nm, need_f32=False):
                    """PSUM -> (hi, lo) bf16 tiles [+ optional f32 tile]."""
                    if enc_mode != "bf16x3":
                        t = av.tile([PCH, RB], f32, tag=nm + "f")
                        nc.scalar.activation(t[:], ps_tile[:], func, bias=bias_ap)
                        return t, t
                    tf = av.tile([PCH, RB], f32, tag=nm + "f")
                    nc.scalar.activation(tf[:], ps_tile[:], func, bias=bias_ap)
                    th = av.tile([PCH, RB], bf16, tag=nm + "h")
                    nc.scalar.activation(th[:], tf[:], AF.Copy)
                    tl = av.tile([PCH, RB], bf16, tag=nm + "l")
                    nc.vector.tensor_tensor(out=tl[:], in0=tf[:], in1=th[:],
                                            op=OP.subtract)
                    return (th, tl), tf

                def wslice(w, rows, cols):
                    # w is tile (f32) or (hi, lo) tile tuple (bf16x3)
                    if enc_mode == "bf16x3":
                        return (w[0][rows, cols], w[1][rows, cols])
                    return w[rows, cols]

                def xap(x):
                    if enc_mode == "bf16x3":
                        return (x[0][:], x[1][:])
                    return x[:]

                def xslc(x, cols):
                    if enc_mode == "bf16x3":
                        return (x[0][:, cols], x[1][:, cols])
                    return x[:, cols]

                full = slice(None)
                h1 = []
                for j in range(2):
                    ps = pm.tile([PCH, RB], f32, tag=f"h1_{j}")
                    jc = slice(j * 128, (j + 1) * 128)
                    emit_group(ps, products(wslice(w1T, full, jc), xap(xT), enc_mode))
                    hs, _ = act_split(ps, AF.Relu, b1t[j][:], f"h1{j}")
                    h1.append(hs)
                h2 = []
                for j in range(2):
                    ps = pm.tile([PCH, RB], f32, tag=f"h2_{j}")
                    jc = slice(j * 128, (j + 1) * 128)
                    pairs = []
                    for k in range(2):
                        pairs += products(wslice(w2T[k], full, jc), xap(h1[k]),
                                          enc_mode)
                    emit_group(ps, pairs)
                    hs, _ = act_split(ps, AF.Relu, b2t[j][:], f"h2{j}")
                    h2.append(hs)
                encps = pm.tile([PCH, RB], f32, tag="enc")
                pairs = []
                for k in range(2):
                    pairs += products(wslice(muT[k], full, full), xap(h2[k]),
                                      enc_mode)
                emit_group(encps, pairs)
                encs, encf = act_split(encps, AF.Identity, bmt[:], "enc",
                                       need_f32=True)
                sq_scr = scr.tile([PCH, RB], f32, tag="sqscr")
                nc.scalar.activation(sq_scr[:], encf[:], AF.Square,
                                     accum_out=esq_st[:, b:b + 1])

                # VQ: score = enc . e_k - 0.5*|e_k|^2 in PSUM; argmax over k
                mv = av.tile([PCH, NCH * 8], f32, tag="mv")   # col layout e*NCH+c
                ib = av.tile([PCH, NCH * 8], u32, tag="ib")
                for c in range(NCH):
                    cs = slice(c * 128, (c + 1) * 128)
                    sps = pm.tile([PCH, KCB], f32, tag="s", bufs=2)
                    if vq_mode == "bf16x3":
                        eh, el = encs[0][:, cs], encs[1][:, cs]
                        pairs = [(eh, embT[0][:]), (eh, embT[1][:]),
                                 (el, embT[0][:])]
                    else:
                        pairs = [(encf[:, cs], embT[:])]
                    emit_group(sps, pairs,
                               extra_last=(ones2_bf[:], bias_hl[:]))
                    mv8 = mv[:].rearrange("p (e c) -> p c e", c=NCH)[:, c, :]
                    ib8 = ib[:].rearrange("p (e c) -> p c e", c=NCH)[:, c, :]
                    nc.vector.max(mv8, sps[:])
                    nc.vector.max_index(ib8, mv8, sps[:])
                nc.vector.tensor_reduce(out=mvr_st[:, b:b + 1], in_=mv[:, 0:NCH],
                                        axis=AX.X, op=OP.add)

                gt = av.tile([PCH, NCH, TW], f32, tag="gath", bufs=3)
                for c in range(NCH):
                    # HW indirect DMA supports one offset per partition
                    nc.gpsimd.indirect_dma_start(
                        out=gt[:, c, :], out_offset=None,
                        in_=table[:],
                        in_offset=bass.IndirectOffsetOnAxis(ap=ib[:, c:c + 1], axis=0),
                    )
                nc.sync.dma_start(
                    q_d[r0:r0 + RB, :].rearrange("(c p) d -> p c d", p=PCH),
                    gt[:, :, 0:D])
                nc.sync.dma_start(
                    idx_d[r0:r0 + RB].rearrange("(c p) -> p c", p=PCH),
                    ib[:, 0:NCH].bitcast(i32))

                t6 = scr.tile([PCH, NCH, A], f32, tag="t6")
                nc.vector.tensor_tensor(out=t6[:], in0=gt[:, :, D:D + A], in1=actn[:],
                                        op=OP.subtract)
                sq6 = scr.tile([PCH, NCH * A], f32, tag="sq6")
                nc.scalar.activation(sq6[:], t6[:].rearrange("p c a -> p (c a)"),
                                     AF.Square, accum_out=rsq_st[:, b:b + 1])

            # ---------- partials ----------
            pc = stg.tile([PCH, 3], f32)
            nc.vector.tensor_reduce(out=pc[:, 0:1], in_=esq_st[:], axis=AX.X, op=OP.add)
            nc.vector.tensor_reduce(out=pc[:, 1:2], in_=mvr_st[:], axis=AX.X, op=OP.add)
            nc.vector.tensor_reduce(out=pc[:, 2:3], in_=rsq_st[:], axis=AX.X, op=OP.add)
            nc.sync.dma_start(par_d[:], pc[:])

    nc.compile()
    return nc


def _make_in_map(core, action, enc1_w, enc1_b, enc2_w, enc2_b, mu_w, mu_b, emb,
                 dec1_w, dec1_b, dec2_w, dec2_b, rec_w, rec_b, bloc=BLOC,
                 enc_mode=ENC_MODE, vq_mode=VQ_MODE):
    c = np.ascontiguousarray
    sl = slice(core * bloc, (core + 1) * bloc)
    act = c(action[sl]).astype(np.float32)
    rT = np.zeros((H, RPAD), np.float32)
    rT[:, :A] = rec_w.T
    br = np.zeros((RPAD,), np.float32)
    br[:A] = rec_b
    m = {
        "act": act,
        "b1": c(enc1_b),
        "b2": c(enc2_b),
        "bm": c(mu_b),
        "embTf": c(emb.T),
        "embN": c(emb),
        "d1T": c(dec1_w.T),
        "bd1": c(dec1_b),
        "d2T": c(dec2_w.T),
        "bd2": c(dec2_b),
        "rT": rT,
        "br": br,
    }
    if enc_mode == "bf16x3":
        hi, lo = _split_bf16(c(act.T))
        m["actTh"], m["actTl"] = c(hi), c(lo)
        for nm, w in [("w1T", enc1_w.T), ("w2T", enc2_w.T), ("muT", mu_w.T)]:
            hi, lo = _split_bf16(c(w.astype(np.float32)))
            m[nm + "h"], m[nm + "l"] = c(hi), c(lo)
    else:
        m["actT"] = c(act.T)
        m["w1T"] = c(enc1_w.T)
        m["w2T"] = c(enc2_w.T)
        m["muT"] = c(mu_w.T)
    if vq_mode == "bf16x3":
        hi, lo = _split_bf16(c(emb.T.astype(np.float32)))
        m["embTh"], m["embTl"] = c(hi), c(lo)
    else:
        m["embT"] = c(emb.T)
    return m


def kernel(action, enc1_w, enc1_b, enc2_w, enc2_b, mu_w, mu_b, emb,
           dec1_w, dec1_b, dec2_w, dec2_b, rec_w, rec_b,
           _trace=False, _result_hook=None):
    from concourse.bass_utils import run_bass_kernel_spmd

    key = (BLOC, ENC_MODE, VQ_MODE)
    if key not in _NC_CACHE:
        _NC_CACHE[key] = _build_nc(BLOC, ENC_MODE, VQ_MODE)
    nc = _NC_CACHE[key]

    args = (action, enc1_w, enc1_b, enc2_w, enc2_b, mu_w, mu_b, emb,
            dec1_w, dec1_b, dec2_w, dec2_b, rec_w, rec_b)
    in_maps = [_make_in_map(core, *args) for core in range(NCORES)]
    res = run_bass_kernel_spmd(nc, in_maps, core_ids=list(range(NCORES)),
                               trace=_trace)
    if _result_hook is not None:
        _result_hook(res)

    idx = np.concatenate([res.results[c]["idx_o"] for c in range(NCORES)])
    q_st = np.concatenate([res.results[c]["q_o"] for c in range(NCORES)])
    par = np.stack([res.results[c]["partials"] for c in range(NCORES)])
    # partials: [:, :, 0]=sum(enc^2), [:, :, 1]=sum(maxval), [:, :, 2]=sum((rec-act)^2)
    esum = float(par[:, :, 0].astype(np.float64).sum())
    msum = float(par[:, :, 1].astype(np.float64).sum())
    rsum = float(par[:, :, 2].astype(np.float64).sum())
    vq_sum = esum - 2.0 * msum          # sum over rows of min squared distance
    recons_loss = rsum / (B_TOTAL * A)
    vq_loss = (1.0 + BETA) * vq_sum / (B_TOTAL * D)
    total_loss = np.float32(recons_loss + vq_loss)
    return idx.astype(np.int32), q_st.astype(np.float32), total_loss
